# revision 7
# baseline (speedup 1.0000x reference)
"""Distributed CLIP loss kernel for Trainium2 (8 NeuronCores).

Strategy (dual-direction sharding):
  loss = 0.5*(mean_i lse_rows(i) + mean_j lse_cols(j)) - mean(diag)
  where logits = scale * (z_schema @ z_seal.T), scale = min(exp(logit_scale), 100).

  Cores 0-3: row direction.  Core m holds z_schema rows [m*4096,(m+1)*4096)
             (pre-scaled by `scale` on host) and full z_seal; computes lse
             over each of its 4096 logits rows.
  Cores 4-7: column direction.  Same program on z_seal strips x full
             z_schema: lse over rows of logits^T == lse over columns.
  Pure SPMD: identical program, different data.  diag partials come from a
  row-major elementwise pass; host uses cores 0-3's copies.

  Per core main loop: stream B^T in column slabs; per 128-row tile and
  2048-col chunk: 8 fp32 matmuls into PSUM; DVE reduce_max(negate) gives
  -max; ACT exp(x + (-max)) with accum_out gives the chunk sumexp.  A
  two-level logsumexp combine over the 8 chunk stats yields each row's
  lse.  Built on bacc (generate_event_semaphores legalizes sync waits to
  the 1-wait-per-instruction ISA limit).
"""

import math

import numpy as np

B = 16384
D = 256
P = 128
KCH = D // P  # 2 k-chunks of 128

# full-size config
STRIP = B // 4  # 4096 rows per core
SLAB = 4096  # columns loaded per B-slab
CHUNK = 2048  # columns per PSUM chunk (4 banks)
MAX_SCALE = 100.0

_CACHE = {}


def build_nc(strip=STRIP, bcols=B, slab=SLAB, chunk=CHUNK):
    """Build the Bass program for one core (SPMD: same program on all)."""
    from contextlib import ExitStack

    import concourse.bacc as bacc
    import concourse.tile as tile
    from concourse import mybir

    f32 = mybir.dt.float32
    AF = mybir.ActivationFunctionType
    AX = mybir.AxisListType
    ALU = mybir.AluOpType

    mi_n = strip // P  # row tiles
    nslabs = bcols // slab
    cps = slab // chunk  # chunks per slab
    nch = nslabs * cps  # chunks per row tile
    nsl = chunk // 512  # matmuls of N=512 per chunk
    dn = min(8, mi_n)  # mi-tiles per diag DMA

    nc = bacc.Bacc()
    a_t = nc.declare_dram_parameter("a_t", [KCH, P, strip], f32, isOutput=False)
    b_t = nc.declare_dram_parameter("b_t", [KCH, P, bcols], f32, isOutput=False)
    # row-major interleave of (scaled A rows, BD rows): [mi, p, 2, D]
    ab_r = nc.declare_dram_parameter("ab_r", [mi_n, P, 2, D], f32, isOutput=False)
    lse_o = nc.declare_dram_parameter("lse", [P, mi_n], f32, isOutput=True)
    diag_o = nc.declare_dram_parameter("diag", [P, mi_n], f32, isOutput=True)

    with tile.TileContext(nc) as tc, ExitStack() as ctx:
        singles = ctx.enter_context(tc.tile_pool(name="singles", bufs=1))
        apool = ctx.enter_context(tc.tile_pool(name="apool", bufs=1))
        dstream = ctx.enter_context(tc.tile_pool(name="dstream", bufs=2))
        bpool = ctx.enter_context(tc.tile_pool(name="bslab", bufs=2))
        psum = ctx.enter_context(tc.tile_pool(name="psum", bufs=2, space="PSUM"))
        epool = ctx.enter_context(tc.tile_pool(name="escratch", bufs=2))
        cpool = ctx.enter_context(tc.tile_pool(name="combine", bufs=1))

        # a strip + diag stream on ACT HWDGE queues; b slabs on SP queues
        a_sb = apool.tile([P, KCH, strip], f32)
        for k in range(KCH):
            nc.scalar.dma_start(out=a_sb[:, k, :], in_=a_t[k])

        # stats (flat 2D so [:, i:i+1] slices are clean (P,1) APs)
        stats_n = singles.tile([P, mi_n * nch], f32)  # -(chunk max)
        stats_s = singles.tile([P, mi_n * nch], f32)  # chunk sumexp

        # ---- main pipeline ----
        for sl in range(nslabs):
            b_sb = bpool.tile([P, KCH, slab], f32)
            for k in range(KCH):
                nc.sync.dma_start(
                    out=b_sb[:, k, :], in_=b_t[k, :, sl * slab : (sl + 1) * slab]
                )
            for mi in range(mi_n):
                for c in range(cps):
                    ps = psum.tile([P, chunk], f32, tag="ps")
                    for k in range(KCH):
                        for n in range(nsl):
                            nc.tensor.matmul(
                                ps[:, n * 512 : (n + 1) * 512],
                                lhsT=a_sb[:, k, mi * P : (mi + 1) * P],
                                rhs=b_sb[:, k, c * chunk + n * 512 : c * chunk + (n + 1) * 512],
                                start=(k == 0),
                                stop=(k == KCH - 1),
                            )
                    idx = mi * nch + sl * cps + c
                    nslot = stats_n[:, idx : idx + 1]
                    nc.vector.reduce_max(out=nslot, in_=ps[:], axis=AX.X, negate=True)
                    e_scr = epool.tile([P, chunk], f32, tag="e")
                    nc.scalar.activation(
                        out=e_scr[:],
                        in_=ps[:],
                        func=AF.Exp,
                        bias=nslot,
                        scale=1.0,
                        accum_out=stats_s[:, idx : idx + 1],
                    )

        # ---- diag partial: diag[p,mi] = sum_d A[mi*P+p,d]*BD[mi*P+p,d] ----
        diag_sb = singles.tile([P, mi_n], f32)
        for g0 in range(0, mi_n, dn):
            t = dstream.tile([P, dn, 2, D], f32)
            nc.scalar.dma_start(
                out=t[:], in_=ab_r[g0 : g0 + dn].rearrange("m p t d -> p m t d")
            )
            for j in range(dn):
                mi = g0 + j
                nc.vector.scalar_tensor_tensor(
                    out=t[:, j, 0, :],
                    in0=t[:, j, 0, :],
                    scalar=1.0,
                    in1=t[:, j, 1, :],
                    op0=ALU.mult,
                    op1=ALU.mult,
                    accum_out=diag_sb[:, mi : mi + 1],
                )
        nc.gpsimd.dma_start(out=diag_o[:], in_=diag_sb[:])

        # ---- combine chunk stats -> lse per row ----
        # n_c = -m_c ; Mn = min_c n_c = -M
        # lse = -Mn + log(sum_c sigma_c * exp(Mn - n_c))
        stats_n3 = stats_n[:].rearrange("p (m c) -> p m c", c=nch)
        stats_s3 = stats_s[:].rearrange("p (m c) -> p m c", c=nch)
        Mn_t = cpool.tile([P, mi_n, 1], f32)
        nc.vector.tensor_reduce(out=Mn_t[:, :, 0], in_=stats_n3, axis=AX.X, op=ALU.min)
        Mn_b = Mn_t[:, :, 0:1].to_broadcast([P, mi_n, nch])
        d_t = cpool.tile([P, mi_n, nch], f32)
        nc.vector.scalar_tensor_tensor(
            out=d_t[:], in0=stats_n3, scalar=-1.0, in1=Mn_b, op0=ALU.mult, op1=ALU.add
        )
        w_t = cpool.tile([P, mi_n, nch], f32)
        nc.scalar.activation(out=w_t[:], in_=d_t[:], func=AF.Exp)
        ws_t = cpool.tile([P, mi_n, nch], f32)
        nc.vector.scalar_tensor_tensor(
            out=ws_t[:], in0=w_t[:], scalar=1.0, in1=stats_s3, op0=ALU.mult, op1=ALU.mult
        )
        S_t = cpool.tile([P, mi_n], f32)
        nc.vector.reduce_sum(out=S_t[:], in_=ws_t[:], axis=AX.X)
        L_t = cpool.tile([P, mi_n], f32)
        nc.scalar.activation(out=L_t[:], in_=S_t[:], func=AF.Ln)
        lse_t = cpool.tile([P, mi_n], f32)
        nc.vector.scalar_tensor_tensor(
            out=lse_t[:],
            in0=Mn_t[:, :, 0],
            scalar=-1.0,
            in1=L_t[:],
            op0=ALU.mult,
            op1=ALU.add,
        )
        nc.gpsimd.dma_start(out=lse_o[:], in_=lse_t[:])

    nc.compile()
    return nc


def _prep_t(x):
    # (N, 256) -> contiguous (2, 128, N) with d on the second axis
    return np.ascontiguousarray(np.asarray(x, np.float32).T).reshape(KCH, P, -1)


def _prep_abr(a_rows_scaled, bd_rows):
    # (strip, D) x2 -> (mi, P, 2, D)
    strip = a_rows_scaled.shape[0]
    out = np.empty((strip, 2, D), np.float32)
    out[:, 0, :] = a_rows_scaled
    out[:, 1, :] = bd_rows
    return out.reshape(strip // P, P, 2, D)


def kernel(z_schema, z_seal, logit_scale):
    from concourse.bass_utils import run_bass_kernel_spmd

    s = np.float32(min(math.exp(float(np.asarray(logit_scale))), MAX_SCALE))
    zs = np.asarray(z_schema, np.float32)
    zl = np.asarray(z_seal, np.float32)
    zsT = _prep_t(zs)
    zlT = _prep_t(zl)

    if "nc" not in _CACHE:
        _CACHE["nc"] = build_nc()
    nc = _CACHE["nc"]

    in_maps = []
    for m in range(8):
        if m < 4:
            AT, BT, base = zsT, zlT, m * STRIP
            Ar, Br = zs, zl
        else:
            AT, BT, base = zlT, zsT, (m - 4) * STRIP
            Ar, Br = zl, zs
        a_scaled_rows = Ar[base : base + STRIP] * s
        in_maps.append(
            {
                "a_t": np.ascontiguousarray(AT[:, :, base : base + STRIP]) * s,
                "b_t": BT,
                "ab_r": _prep_abr(a_scaled_rows, Br[base : base + STRIP]),
            }
        )

    res = run_bass_kernel_spmd(nc, in_maps, list(range(8))).results

    lse_r = np.concatenate([res[m]["lse"].T.ravel() for m in range(4)])
    lse_c = np.concatenate([res[m]["lse"].T.ravel() for m in range(4, 8)])
    diag = np.concatenate([res[m]["diag"].T.ravel() for m in range(4)])
    loss = 0.5 * (lse_r.mean(dtype=np.float64) + lse_c.mean(dtype=np.float64)) - diag.mean(
        dtype=np.float64
    )
    out = np.asarray(loss, dtype=np.float32)
    return (out, out)
